# revision 50
# baseline (speedup 1.0000x reference)
"""MoE (top-2 of 4 experts) Trainium2 kernel.

Strategy (expert-parallel "all-to-all" done host-side):
  - Host computes the tiny gate (N x 4 logits), top-2 routing and softmax
    combine weights with jax-on-CPU (bitwise-identical to the reference
    routing), then dispatches each expert's tokens to 2 of the 8 cores.
  - Each NeuronCore runs a dense 2-layer MLP (x @ w1.T -> gelu -> @ w2.T)
    for ONE expert on its ~4096 assigned tokens, in bf16 matmuls with
    fp32 PSUM accumulation; gelu (tanh approximation, matching the
    reference exactly) runs on the scalar engine.
  - Host scatter-adds the per-expert outputs back with the combine
    weights (each expert's token list has unique indices, so fancy-index
    "+=" per expert is exact).

Device layout is feature-major ("transposed"): activations are [feature,
token] so both matmuls contract over the partition dimension with weights
stationary and tokens as the moving free dimension.
"""

import numpy as np

E, TOPK = 4, 2
N_CORES = 8

_KERNEL_CACHE = {}


def _round_up(x, m):
    return (x + m - 1) // m * m


def _groups(C):
    """Token groups, each <=512 (one fp32 PSUM bank per matmul output).

    Near-equal sizes (multiples of 32) amortize fixed per-matmul issue cost
    better than 512-groups plus a tiny remainder. Returns [(off, glen)].
    """
    n_groups = (C + 511) // 512
    base = C // n_groups // 32 * 32
    rem = C - base * n_groups
    sizes = [base + 32 if i < rem // 32 else base for i in range(n_groups)]
    assert sum(sizes) == C and all(s <= 512 for s in sizes)
    out = []
    off = 0
    for glen in sizes:
        out.append((off, glen))
        off += glen
    return out


def _build_moe_bass(C):
    """Bass program for one core: dense 1-expert MLP over C tokens.

    Inputs (per core):
      xg  [G, 128, 4, 512] bf16  x^T group-major: xg[g,p,k,t] = x^T[128k+p, off_g+t]
      w1t [4, 128, 2048] bf16  w1[e]^T k-tiles ([D, H] k-major)
      w2t [16, 128, 512] bf16  w2[e]^T k-tiles ([H, D] k-major)
      b1c [128, 16] f32        b1[e] chunk-major (column c = b1[128c:128c+128])
      b2c [128, 4]  f32        b2[e] chunk-major
    Output:
      og  [4, 128, C]   f32    out^T k-tiles (D on partitions)
    """
    import concourse.bacc as bacc
    import concourse.tile as tile
    from concourse import mybir

    nc = bacc.Bacc("TRN2", target_bir_lowering=False, debug=False)

    n_groups_total = (C + 511) // 512
    xg_h = nc.dram_tensor(
        "xg", [n_groups_total, 128, 4, 512], mybir.dt.bfloat16, kind="ExternalInput"
    )
    w1_h = nc.dram_tensor("w1t", [4, 128, 2048], mybir.dt.bfloat16, kind="ExternalInput")
    w2_h = nc.dram_tensor("w2t", [16, 128, 512], mybir.dt.bfloat16, kind="ExternalInput")
    b1_h = nc.dram_tensor("b1c", [128, 16], mybir.dt.float32, kind="ExternalInput")
    b2_h = nc.dram_tensor("b2c", [128, 4], mybir.dt.float32, kind="ExternalInput")
    og_h = nc.dram_tensor("og", [4, 128, C], mybir.dt.float32, kind="ExternalOutput")

    GELU = mybir.ActivationFunctionType.Gelu_apprx_tanh

    groups = _groups(C)

    with tile.TileContext(nc) as tc:
        with (
            tc.tile_pool(name="weights", bufs=1) as wpool,
            tc.tile_pool(name="xin", bufs=2) as xpool,
            tc.tile_pool(name="bias", bufs=1) as bpool,
            tc.tile_pool(name="warm", bufs=1) as warmpool,
            tc.tile_pool(name="hs", bufs=4) as hpool,
            tc.tile_pool(name="oevac", bufs=4) as opool,
            tc.tile_pool(name="ps1", bufs=4, space="PSUM") as ps1,
            tc.tile_pool(name="ps2", bufs=4, space="PSUM") as ps2,
        ):
            # PE warmup: dummy matmuls on a memset tile keep TensorE busy
            # (and the HAM clock-gate warming) during the initial DMA wait.
            warm = warmpool.tile([128, 640], mybir.dt.bfloat16)
            nc.gpsimd.memset(warm[:], 0)
            # Warmup PSUM comes from ps2's pool so its bank is reused once
            # real work starts (8 banks total: 4 ps1 + 4 ps2).
            wps = ps2.tile([128, 512], mybir.dt.float32, tag="po")
            for _ in range(8):
                nc.tensor.matmul(
                    wps[:], warm[:, :128], warm[:, 128:640], start=True, stop=True
                )

            # w1 in four quarter-H tiles so the first 4 H-chunks of group 0
            # can start as soon as the first 0.5 MB lands.
            w1q = [
                wpool.tile(
                    [128, 4, 512],
                    mybir.dt.bfloat16,
                    name=f"w1q{i}",
                    tag=f"w1q{i}",
                )
                for i in range(4)
            ]
            w2s = wpool.tile([128, 16, 512], mybir.dt.bfloat16)
            b1s = bpool.tile([128, 16], mybir.dt.float32)
            b2s = bpool.tile([128, 4], mybir.dt.float32)

            # DMA queues: the scalar queue's HW ring starts ~1.8us earlier
            # than sync's (measured), so it carries the critical first loads
            # (w1 slab 0, then x group 0) followed by the remaining x groups
            # (paced by the xin pool, bufs=2). sync carries w1 slabs 1-3 then
            # w2 (FIFO; done before MM2 of group 0 needs it). gpsimd carries
            # biases + outputs.
            def dma_w1(engine, i):
                engine.dma_start(
                    w1q[i][:],
                    w1_h.ap()[:, :, i * 512 : (i + 1) * 512].rearrange(
                        "k p h -> p k h"
                    ),
                )

            dma_w1(nc.scalar, 0)
            for i in range(1, 4):
                dma_w1(nc.sync, i)
            nc.sync.dma_start(w2s[:], w2_h.ap().rearrange("k p h -> p k h"))
            nc.gpsimd.dma_start(b1s[:], b1_h.ap())
            nc.gpsimd.dma_start(b2s[:], b2_h.ap())
            xtiles = []
            for g, (off, glen) in enumerate(groups):
                xsg = xpool.tile([128, 4, 512], mybir.dt.bfloat16, tag="xsg")
                nc.scalar.dma_start(xsg[:], xg_h.ap()[g])
                xtiles.append(xsg)

            for g, (off, glen) in enumerate(groups):
                xsg = xtiles[g]
                # ---- h^T = gelu(w1^T.T @ x^T + b1): 16 H-chunks of 128 ----
                hs = hpool.tile([128, 16, 512], mybir.dt.bfloat16)
                for chunk in range(16):
                    w1t = w1q[chunk // 4]
                    c0 = (chunk % 4) * 128
                    ps = ps1.tile([128, 512], mybir.dt.float32)
                    for k in range(4):
                        nc.tensor.matmul(
                            ps[:, :glen],
                            w1t[:, k, c0 : c0 + 128],
                            xsg[:, k, :glen],
                            start=(k == 0),
                            stop=(k == 3),
                        )
                    nc.scalar.activation(
                        hs[:, chunk, :glen],
                        ps[:, :glen],
                        GELU,
                        bias=b1s[:, chunk : chunk + 1],
                    )
                # ---- out^T = w2^T.T @ h^T + b2: 4 D-chunks of 128 ----
                last_group = g == len(groups) - 1
                ot = opool.tile([128, 4, 512], mybir.dt.float32)
                for dc in range(4):
                    po = ps2.tile([128, 512], mybir.dt.float32, tag="po")
                    for k2 in range(16):
                        nc.tensor.matmul(
                            po[:, :glen],
                            w2s[:, k2, dc * 128 : (dc + 1) * 128],
                            hs[:, k2, :glen],
                            start=(k2 == 0),
                            stop=(k2 == 15),
                        )
                    nc.vector.tensor_scalar_add(
                        ot[:, dc, :glen], po[:, :glen], b2s[:, dc : dc + 1]
                    )
                    if last_group:
                        # Per-chunk stores so the final writes overlap the
                        # last matmuls instead of serializing into the tail.
                        nc.gpsimd.dma_start(
                            og_h.ap()[dc, :, off : off + glen], ot[:, dc, :glen]
                        )
                if not last_group:
                    nc.gpsimd.dma_start(
                        og_h.ap()[:, :, off : off + glen].rearrange("d p t -> p d t"),
                        ot[:, :, :glen],
                    )

    nc.compile()
    return nc


def _routing(x, gate_w):
    """Replicate the reference's routing bitwise using jax on CPU.

    Returns (cw_idx [N, 2] int32, cw_w [N, 2] f32, balancing_loss f32).
    """
    import jax
    import jax.numpy as jnp

    cpu = jax.devices("cpu")[0]
    with jax.default_device(cpu):
        xf = jnp.asarray(x).reshape(-1, x.shape[-1])
        gw = jnp.asarray(gate_w)
        gate = xf @ gw.T
        m = jnp.mean(gate)
        balancing_loss = jnp.sum(m * jnp.log(m + 0.1))
        w, idx = jax.lax.top_k(gate, TOPK)
        w = jax.nn.softmax(w.astype(jnp.float32), axis=1)
        return (
            np.asarray(idx, dtype=np.int32),
            np.asarray(w, dtype=np.float32),
            np.asarray(balancing_loss, dtype=np.float32),
        )


def kernel(x, gate_w, w1, b1, w2, b2):
    import ml_dtypes

    from concourse import bass_utils

    bf16 = ml_dtypes.bfloat16

    x = np.asarray(x, dtype=np.float32)
    gate_w = np.asarray(gate_w, dtype=np.float32)
    w1 = np.asarray(w1, dtype=np.float32)
    b1 = np.asarray(b1, dtype=np.float32)
    w2 = np.asarray(w2, dtype=np.float32)
    b2 = np.asarray(b2, dtype=np.float32)

    n = x.shape[0] * x.shape[1]
    d = x.shape[2]
    xf = x.reshape(n, d)

    idx, w, balancing_loss = _routing(x, gate_w)

    # Token lists per expert; split each across 2 cores.
    expert_rows = []  # per core: (expert, token_idx array, weight array)
    for e in range(E):
        sel = np.nonzero(idx == e)  # (token, slot) pairs, token-ascending
        toks = sel[0].astype(np.int64)
        wts = w[sel[0], sel[1]]
        half = (len(toks) + 1) // 2
        expert_rows.append((e, toks[:half], wts[:half]))
        expert_rows.append((e, toks[half:], wts[half:]))

    counts = [len(t) for (_, t, _) in expert_rows]
    C = max(128, _round_up(max(counts), 32))

    if C not in _KERNEL_CACHE:
        _KERNEL_CACHE[C] = _build_moe_bass(C)
    nc = _KERNEL_CACHE[C]

    # Per-core host marshalling. xg is group-major so each group's DMA is
    # one fully contiguous [128, 4*512] transfer.
    groups = _groups(C)
    in_maps = []
    for e, toks, _wts in expert_rows:
        xg = np.zeros((512, C), dtype=bf16)
        xg[:, : len(toks)] = xf[toks].T.astype(bf16)
        xgk = xg.reshape(4, 128, C)
        xg_grp = np.zeros((len(groups), 128, 4, 512), dtype=bf16)
        for g, (off, glen) in enumerate(groups):
            xg_grp[g, :, :, :glen] = xgk[:, :, off : off + glen].transpose(1, 0, 2)
        in_maps.append(
            {
                "xg": xg_grp,
                "w1t": np.ascontiguousarray(w1[e].T.astype(bf16).reshape(4, 128, 2048)),
                "w2t": np.ascontiguousarray(w2[e].T.astype(bf16).reshape(16, 128, 512)),
                "b1c": np.ascontiguousarray(b1[e].reshape(16, 128).T),
                "b2c": np.ascontiguousarray(b2[e].reshape(4, 128).T),
            }
        )

    res = bass_utils.run_bass_kernel_spmd(
        nc, in_maps, core_ids=list(range(N_CORES)), trace=False
    )

    out = np.zeros((n, d), dtype=np.float32)
    for (e, toks, wts), r in zip(expert_rows, res.results):
        og = r["og"].reshape(512, C)  # [D, C]
        out[toks] += og[:, : len(toks)].T * wts[:, None]

    return out.reshape(x.shape), balancing_loss


# revision 51
# speedup vs baseline: 1.0153x; 1.0153x over previous
"""MoE (top-2 of 4 experts) Trainium2 kernel.

Strategy (expert-parallel "all-to-all" done host-side):
  - Host computes the tiny gate (N x 4 logits), top-2 routing and softmax
    combine weights with jax-on-CPU (bitwise-identical to the reference
    routing), then dispatches each expert's tokens to 2 of the 8 cores.
  - Each NeuronCore runs a dense 2-layer MLP (x @ w1.T -> gelu -> @ w2.T)
    for ONE expert on its ~4096 assigned tokens, in bf16 matmuls with
    fp32 PSUM accumulation; gelu (tanh approximation, matching the
    reference exactly) runs on the scalar engine.
  - Host scatter-adds the per-expert outputs back with the combine
    weights (each expert's token list has unique indices, so fancy-index
    "+=" per expert is exact).

Device layout is feature-major ("transposed"): activations are [feature,
token] so both matmuls contract over the partition dimension with weights
stationary and tokens as the moving free dimension.
"""

import numpy as np

E, TOPK = 4, 2
N_CORES = 8

_KERNEL_CACHE = {}


def _round_up(x, m):
    return (x + m - 1) // m * m


def _groups(C):
    """Token groups, each <=512 (one fp32 PSUM bank per matmul output).

    Near-equal sizes (multiples of 32) amortize fixed per-matmul issue cost
    better than 512-groups plus a tiny remainder. Returns [(off, glen)].
    """
    n_groups = (C + 511) // 512
    base = C // n_groups // 32 * 32
    rem = C - base * n_groups
    sizes = [base + 32 if i < rem // 32 else base for i in range(n_groups)]
    assert sum(sizes) == C and all(s <= 512 for s in sizes)
    out = []
    off = 0
    for glen in sizes:
        out.append((off, glen))
        off += glen
    return out


def _build_moe_bass(C):
    """Bass program for one core: dense 1-expert MLP over C tokens.

    Inputs (per core):
      xg  [G, 128, 4, 512] bf16  x^T group-major: xg[g,p,k,t] = x^T[128k+p, off_g+t]
      w1t [4, 128, 2048] bf16  w1[e]^T k-tiles ([D, H] k-major)
      w2t [16, 128, 512] bf16  w2[e]^T k-tiles ([H, D] k-major)
      b1c [128, 16] f32        b1[e] chunk-major (column c = b1[128c:128c+128])
      b2c [128, 4]  f32        b2[e] chunk-major
    Output:
      og  [4, 128, C]   f32    out^T k-tiles (D on partitions)
    """
    import concourse.bacc as bacc
    import concourse.tile as tile
    from concourse import mybir

    nc = bacc.Bacc("TRN2", target_bir_lowering=False, debug=False)

    n_groups_total = (C + 511) // 512
    xg_h = nc.dram_tensor(
        "xg", [n_groups_total, 128, 4, 512], mybir.dt.bfloat16, kind="ExternalInput"
    )
    w1_h = nc.dram_tensor("w1t", [4, 128, 2048], mybir.dt.bfloat16, kind="ExternalInput")
    w2_h = nc.dram_tensor("w2t", [16, 128, 512], mybir.dt.bfloat16, kind="ExternalInput")
    b1_h = nc.dram_tensor("b1c", [128, 16], mybir.dt.float32, kind="ExternalInput")
    b2_h = nc.dram_tensor("b2c", [128, 4], mybir.dt.float32, kind="ExternalInput")
    og_h = nc.dram_tensor("og", [4, 128, C], mybir.dt.float32, kind="ExternalOutput")

    GELU = mybir.ActivationFunctionType.Gelu_apprx_tanh

    groups = _groups(C)

    with tile.TileContext(nc) as tc:
        with (
            tc.tile_pool(name="weights", bufs=1) as wpool,
            tc.tile_pool(name="xin", bufs=2) as xpool,
            tc.tile_pool(name="bias", bufs=1) as bpool,
            tc.tile_pool(name="warm", bufs=1) as warmpool,
            tc.tile_pool(name="hs", bufs=4) as hpool,
            tc.tile_pool(name="oevac", bufs=4) as opool,
            tc.tile_pool(name="ps1", bufs=4, space="PSUM") as ps1,
            tc.tile_pool(name="ps2", bufs=4, space="PSUM") as ps2,
        ):
            # PE warmup: dummy matmuls on a memset tile keep TensorE busy
            # (and the HAM clock-gate warming) during the initial DMA wait.
            warm = warmpool.tile([128, 640], mybir.dt.bfloat16)
            nc.gpsimd.memset(warm[:], 0)
            # Warmup PSUM comes from ps2's pool so its bank is reused once
            # real work starts (8 banks total: 4 ps1 + 4 ps2).
            wps = ps2.tile([128, 512], mybir.dt.float32, tag="po")
            for _ in range(8):
                nc.tensor.matmul(
                    wps[:], warm[:, :128], warm[:, 128:640], start=True, stop=True
                )

            # w1 in four quarter-H tiles so the first 4 H-chunks of group 0
            # can start as soon as the first 0.5 MB lands.
            w1q = [
                wpool.tile(
                    [128, 4, 512],
                    mybir.dt.bfloat16,
                    name=f"w1q{i}",
                    tag=f"w1q{i}",
                )
                for i in range(4)
            ]
            w2s = wpool.tile([128, 16, 512], mybir.dt.bfloat16)
            b1s = bpool.tile([128, 16], mybir.dt.float32)
            b2s = bpool.tile([128, 4], mybir.dt.float32)

            # DMA queues: sync carries w1 then w2 (w1 is the critical first
            # load; w2 follows FIFO, done before MM2 of group 0 needs it).
            # scalar carries x groups, paced by the xin pool (bufs=2) so
            # early HBM bandwidth goes to w1. gpsimd carries biases + outputs.
            for i in range(4):
                nc.sync.dma_start(
                    w1q[i][:],
                    w1_h.ap()[:, :, i * 512 : (i + 1) * 512].rearrange(
                        "k p h -> p k h"
                    ),
                )
            nc.sync.dma_start(w2s[:], w2_h.ap().rearrange("k p h -> p k h"))
            nc.gpsimd.dma_start(b1s[:], b1_h.ap())
            nc.gpsimd.dma_start(b2s[:], b2_h.ap())
            xtiles = []
            for g, (off, glen) in enumerate(groups):
                xsg = xpool.tile([128, 4, 512], mybir.dt.bfloat16, tag="xsg")
                nc.scalar.dma_start(xsg[:], xg_h.ap()[g])
                xtiles.append(xsg)

            for g, (off, glen) in enumerate(groups):
                xsg = xtiles[g]
                # ---- h^T = gelu(w1^T.T @ x^T + b1): 16 H-chunks of 128 ----
                hs = hpool.tile([128, 16, 512], mybir.dt.bfloat16)
                for chunk in range(16):
                    w1t = w1q[chunk // 4]
                    c0 = (chunk % 4) * 128
                    ps = ps1.tile([128, 512], mybir.dt.float32)
                    for k in range(4):
                        nc.tensor.matmul(
                            ps[:, :glen],
                            w1t[:, k, c0 : c0 + 128],
                            xsg[:, k, :glen],
                            start=(k == 0),
                            stop=(k == 3),
                        )
                    nc.scalar.activation(
                        hs[:, chunk, :glen],
                        ps[:, :glen],
                        GELU,
                        bias=b1s[:, chunk : chunk + 1],
                    )
                # ---- out^T = w2^T.T @ h^T + b2: 4 D-chunks of 128 ----
                last_group = g == len(groups) - 1
                ot = opool.tile([128, 4, 512], mybir.dt.float32)
                for dc in range(4):
                    po = ps2.tile([128, 512], mybir.dt.float32, tag="po")
                    for k2 in range(16):
                        nc.tensor.matmul(
                            po[:, :glen],
                            w2s[:, k2, dc * 128 : (dc + 1) * 128],
                            hs[:, k2, :glen],
                            start=(k2 == 0),
                            stop=(k2 == 15),
                        )
                    nc.vector.tensor_scalar_add(
                        ot[:, dc, :glen], po[:, :glen], b2s[:, dc : dc + 1]
                    )
                    if last_group:
                        # Per-chunk stores so the final writes overlap the
                        # last matmuls instead of serializing into the tail.
                        nc.gpsimd.dma_start(
                            og_h.ap()[dc, :, off : off + glen], ot[:, dc, :glen]
                        )
                if not last_group:
                    nc.gpsimd.dma_start(
                        og_h.ap()[:, :, off : off + glen].rearrange("d p t -> p d t"),
                        ot[:, :, :glen],
                    )

    nc.compile()
    return nc


def _routing(x, gate_w):
    """Replicate the reference's routing bitwise using jax on CPU.

    Returns (cw_idx [N, 2] int32, cw_w [N, 2] f32, balancing_loss f32).
    """
    import jax
    import jax.numpy as jnp

    cpu = jax.devices("cpu")[0]
    with jax.default_device(cpu):
        xf = jnp.asarray(x).reshape(-1, x.shape[-1])
        gw = jnp.asarray(gate_w)
        gate = xf @ gw.T
        m = jnp.mean(gate)
        balancing_loss = jnp.sum(m * jnp.log(m + 0.1))
        w, idx = jax.lax.top_k(gate, TOPK)
        w = jax.nn.softmax(w.astype(jnp.float32), axis=1)
        return (
            np.asarray(idx, dtype=np.int32),
            np.asarray(w, dtype=np.float32),
            np.asarray(balancing_loss, dtype=np.float32),
        )


def kernel(x, gate_w, w1, b1, w2, b2):
    import ml_dtypes

    from concourse import bass_utils

    bf16 = ml_dtypes.bfloat16

    x = np.asarray(x, dtype=np.float32)
    gate_w = np.asarray(gate_w, dtype=np.float32)
    w1 = np.asarray(w1, dtype=np.float32)
    b1 = np.asarray(b1, dtype=np.float32)
    w2 = np.asarray(w2, dtype=np.float32)
    b2 = np.asarray(b2, dtype=np.float32)

    n = x.shape[0] * x.shape[1]
    d = x.shape[2]
    xf = x.reshape(n, d)

    idx, w, balancing_loss = _routing(x, gate_w)

    # Token lists per expert; split each across 2 cores.
    expert_rows = []  # per core: (expert, token_idx array, weight array)
    for e in range(E):
        sel = np.nonzero(idx == e)  # (token, slot) pairs, token-ascending
        toks = sel[0].astype(np.int64)
        wts = w[sel[0], sel[1]]
        half = (len(toks) + 1) // 2
        expert_rows.append((e, toks[:half], wts[:half]))
        expert_rows.append((e, toks[half:], wts[half:]))

    counts = [len(t) for (_, t, _) in expert_rows]
    C = max(128, _round_up(max(counts), 32))

    if C not in _KERNEL_CACHE:
        _KERNEL_CACHE[C] = _build_moe_bass(C)
    nc = _KERNEL_CACHE[C]

    # Per-core host marshalling. xg is group-major so each group's DMA is
    # one fully contiguous [128, 4*512] transfer.
    groups = _groups(C)
    in_maps = []
    for e, toks, _wts in expert_rows:
        xg = np.zeros((512, C), dtype=bf16)
        xg[:, : len(toks)] = xf[toks].T.astype(bf16)
        xgk = xg.reshape(4, 128, C)
        xg_grp = np.zeros((len(groups), 128, 4, 512), dtype=bf16)
        for g, (off, glen) in enumerate(groups):
            xg_grp[g, :, :, :glen] = xgk[:, :, off : off + glen].transpose(1, 0, 2)
        in_maps.append(
            {
                "xg": xg_grp,
                "w1t": np.ascontiguousarray(w1[e].T.astype(bf16).reshape(4, 128, 2048)),
                "w2t": np.ascontiguousarray(w2[e].T.astype(bf16).reshape(16, 128, 512)),
                "b1c": np.ascontiguousarray(b1[e].reshape(16, 128).T),
                "b2c": np.ascontiguousarray(b2[e].reshape(4, 128).T),
            }
        )

    res = bass_utils.run_bass_kernel_spmd(
        nc, in_maps, core_ids=list(range(N_CORES)), trace=False
    )

    out = np.zeros((n, d), dtype=np.float32)
    for (e, toks, wts), r in zip(expert_rows, res.results):
        og = r["og"].reshape(512, C)  # [D, C]
        out[toks] += og[:, : len(toks)].T * wts[:, None]

    return out.reshape(x.shape), balancing_loss
